# revision 21
# baseline (speedup 1.0000x reference)
"""Trainium2 Bass kernel for nn_Better_Transformer (block-diag MLP + BatchNorm + tanh ×2).

  o1 = tanh(BN(x @ blockdiag(w1) + b1))
  o3 = tanh(BN(o1 @ blockdiag(w2) + b2 + x))

Strategy (8 NeuronCores, data-parallel over the batch dim):
  - Each core owns 2048 of the 16384 rows; weights/BN params replicated.
  - Feature-major layout on chip ([128 features, rows]); host pre-transposes
    x per core; output returns feature-major and the host transposes back.
  - bias1/bias2 cancel inside BatchNorm and never reach the device.
  - BN1 statistics: 20 blocks via the gram trick on the TensorEngine
    (C_p = x_pT x_p with a fused ones column from an fp8 row-major copy of
    x, then mean = W1T Sx / B and E[y^2] = diag(W1T C W1) / B), 10 blocks
    via ScalarE Copy/Square+accum (ScalarE is otherwise idle in stage A),
    2 via DVE bn_stats.
  - The residual (+x), the u = o2+x materialization, and the BN2 mean
    accumulate in ONE DVE scalar_tensor_tensor per half-block; a second
    STT (u*u) accumulates E[u^2] (11 blocks use ACT Square+accum instead).
  - BN affine rstd: ACT Sqrt for BN1 (before any Tanh -> one table swap
    total), 2-step Newton on DVE for BN2 (keeps ACT in exp_and_others).
  - BN affine + tanh fuse into one ScalarEngine activation per tile.
  - PSUM pools are scoped per stage: stage B gets all 8 banks (bufs=4)
    for deeper PE run-ahead.
"""

import os
import sys
import types

import numpy as np
import ml_dtypes

B, F, P, D = 16384, 4096, 32, 128
NCORES = 8
BC = B // NCORES          # 2048 rows per core
NH = 2                    # [D, 1024] PSUM halves per block
HW = 1024                 # half width
EPS = 1e-5

# stats1 engine split
GRAM_BLOCKS = [0, 1, 2, 4, 5, 8, 9, 16, 17, 18, 20, 21, 24, 25]   # 14 on PE
ACT_BLOCKS: list = []
DVE_BLOCKS = [p for p in range(P) if p not in GRAM_BLOCKS]        # 18 on DVE
# stats2: these blocks' sum(u^2) runs on ACT (Square+accum), rest on DVE STT.
SQ_ACT = [1, 10, 16, 25, 28]

GRAM_A = [p for p in GRAM_BLOCKS if p < 16]
DVE_A = [p for p in DVE_BLOCKS if p < 16]
ACT_A = [p for p in ACT_BLOCKS if p < 16]
GRAM_B = [p for p in GRAM_BLOCKS if p >= 16]
DVE_B = [p for p in DVE_BLOCKS if p >= 16]
ACT_B = [p for p in ACT_BLOCKS if p >= 16]
GROUPED = GRAM_A + DVE_A + ACT_A + GRAM_B + DVE_B + ACT_B
COL1 = {p: i for i, p in enumerate(GROUPED)}
NGA, NDA, NAA = len(GRAM_A), len(DVE_A), len(ACT_A)
NGB, NDB, NAB = len(GRAM_B), len(DVE_B), len(ACT_B)
GCOL = {p: i for i, p in enumerate(GRAM_A + GRAM_B)}   # col in psMean
NG = len(GRAM_BLOCKS)
# stage-C segments: quarters, with the last quarter split into eighths
SEGS = [(0, 8), (8, 16), (16, 24), (24, 28), (28, 32)]

_BF16 = ml_dtypes.bfloat16
_FP8 = ml_dtypes.float8_e4m3fn

_state: dict = {}


def _install_tile_drain_patch():
    """This walrus build rejects >1 sem wait per instruction ("Too many
    sync wait commands" in setupSyncWait).  1) split the end-of-kernel
    drain waits across single-wait NOPs; 2) after assign_waits, hoist
    extra per-instruction waits onto nofuse NOPs."""
    if _state.get("patched"):
        return
    _state["patched"] = True
    import concourse.mybir as mybir
    import concourse.tile as tile_mod
    from concourse.tile import TileContext
    from concourse.vector_clock import ScopedClock, VectorClock

    def _drain_and_barrier(self, tick_clock, wait_clock):
        gc = tick_clock.global_clock
        for i in range(len(gc)):
            if gc[i] > 0:
                c = VectorClock()
                c.require_at_least(i, gc[i])
                nop = self.nc.sync.nop(nofuse=True, hint="tile_exit_wait")
                wait_clock.add_sem_waits(nop.ins, ScopedClock({None: c}))
        self.nc.sync.drain()
        self.nc.all_engine_barrier()
        assert self.sems is not None
        popped = self.nc._tile_sem_poison_stack.pop()
        assert popped is self._sem_poison
        self.nc.clear_and_free_semaphores(list(self.sems.allocated().values()))
        self.nc.all_engine_barrier()

    TileContext._drain_and_barrier = _drain_and_barrier

    _RealWait = tile_mod.TileClockWait

    class _WaitSplitClockWait:
        def __init__(self, tc, ordered):
            self._w = _RealWait(tc, ordered)
            self._tc = tc
            self._ordered = ordered

        def assign_waits(self, bb_name):
            r = self._w.assign_waits(bb_name)
            nc = self._tc.nc
            for insts in self._ordered.values():
                out = []
                for inst in insts:
                    si = inst.sync_info
                    if si is not None and si.on_wait and len(si.on_wait) > 1:
                        waits = list(si.on_wait)
                        for w in waits[:-1]:
                            nop = mybir.InstNoOp(
                                name=nc.get_next_instruction_name(),
                                engine=inst.engine, ins=[], outs=[],
                            )
                            nop.bass_nofuse = True
                            nop.sync_info = mybir.SyncInfo(on_wait=[w], on_update=[])
                            out.append(nop)
                        si.on_wait = [waits[-1]]
                    out.append(inst)
                insts[:] = out
            return r

        def __getattr__(self, k):
            return getattr(self._w, k)

    tile_mod.TileClockWait = _WaitSplitClockWait


def _install_ntff_hook():
    """Optional: lets BASS_TRACE=1 produce an NTFF profile under axon when
    the image's antenv lacks axon_hooks.  Safe no-op on any failure."""
    if "antenv.axon_hooks" in sys.modules:
        return
    try:
        import contextlib
        import ctypes

        so_path = "/opt/axon/libaxon_pjrt.so"
        if not os.path.exists(so_path):
            return
        lib = ctypes.CDLL(so_path)
        if not hasattr(lib, "axon_start_nrt_profile"):
            return
        lib.axon_start_nrt_profile.argtypes = [ctypes.POINTER(ctypes.c_int64), ctypes.c_size_t]
        lib.axon_start_nrt_profile.restype = ctypes.c_int64
        lib.axon_stop_nrt_profile.argtypes = [ctypes.c_char_p]
        lib.axon_stop_nrt_profile.restype = ctypes.c_int64

        @contextlib.contextmanager
        def _hook(output_dir, device_ids):
            import jax
            jax.devices()
            if device_ids:
                ids = (ctypes.c_int64 * len(device_ids))(*device_ids)
                rc = lib.axon_start_nrt_profile(ids, len(device_ids))
            else:
                rc = lib.axon_start_nrt_profile(None, 0)
            if rc != 0:
                raise RuntimeError(f"axon_start_nrt_profile rc={rc}")
            try:
                yield
            finally:
                n = lib.axon_stop_nrt_profile(str(output_dir).encode())
                if n <= 0:
                    print(f"ntff profile: {n} files written", file=sys.stderr)

        mod = types.ModuleType("antenv.axon_hooks")
        mod.get_axon_ntff_profile_hook = lambda: _hook
        mod.set_axon_ntff_profile_hook = lambda h: None
        sys.modules["antenv.axon_hooks"] = mod
    except Exception:
        pass


def _build():
    import concourse.bass as bass
    import concourse.mybir as mybir
    import concourse.tile as tile

    f32 = mybir.dt.float32
    bf16 = mybir.dt.bfloat16
    fp8 = mybir.dt.float8e4
    Tanh = mybir.ActivationFunctionType.Tanh
    Copy = mybir.ActivationFunctionType.Copy
    Square = mybir.ActivationFunctionType.Square
    Sqrt = mybir.ActivationFunctionType.Sqrt
    mult = mybir.AluOpType.mult
    add = mybir.AluOpType.add
    subtract = mybir.AluOpType.subtract
    AX = mybir.AxisListType.X

    nc = bass.Bass(trn_type="TRN2", num_devices=NCORES)

    xt = nc.dram_tensor("xt", [F, BC], bf16, kind="ExternalInput")
    xr = nc.dram_tensor("xr", [BC, NG * D], fp8, kind="ExternalInput")
    w1 = nc.dram_tensor("w1", [D, F], bf16, kind="ExternalInput")
    w2 = nc.dram_tensor("w2", [D, F], bf16, kind="ExternalInput")
    g1 = nc.dram_tensor("g1", [D, P], f32, kind="ExternalInput")   # grouped col order
    bt1 = nc.dram_tensor("bt1", [D, P], f32, kind="ExternalInput")  # grouped col order
    g3 = nc.dram_tensor("g3", [D, P], f32, kind="ExternalInput")   # natural order
    bt3 = nc.dram_tensor("bt3", [D, P], f32, kind="ExternalInput")
    out = nc.dram_tensor("out", [F, BC], bf16, kind="ExternalOutput")

    with tile.TileContext(nc) as tc:
        with (
            tc.tile_pool(name="const", bufs=1) as const,
            tc.tile_pool(name="xup", bufs=1) as xup,
            tc.tile_pool(name="xrp", bufs=5) as xrp,
            tc.tile_pool(name="stat", bufs=1) as statp,
            tc.tile_pool(name="csb", bufs=2) as csbp,
            tc.tile_pool(name="o1p", bufs=3) as o1p,
            tc.tile_pool(name="scrp", bufs=2) as scrp,
            tc.tile_pool(name="ofp", bufs=3) as ofp,
            tc.tile_pool(name="dram", bufs=1, space="DRAM") as dram,
        ):
            w1_sb = const.tile([D, F], bf16)
            w2_sb = const.tile([D, F], bf16)
            g1_sb = const.tile([D, P], f32)
            bt1_sb = const.tile([D, P], f32)
            g3_sb = const.tile([D, P], f32)
            bt3_sb = const.tile([D, P], f32)
            ones_sb = const.tile([D, 1], bf16)
            eps_sb = const.tile([D, 1], f32)
            nc.sync.dma_start(w1_sb, w1[:])
            nc.sync.dma_start(w2_sb, w2[:])
            nc.sync.dma_start(g1_sb, g1[:])
            nc.sync.dma_start(bt1_sb, bt1[:])
            nc.sync.dma_start(g3_sb, g3[:])
            nc.sync.dma_start(bt3_sb, bt3[:])
            nc.vector.memset(ones_sb, 1.0)
            nc.vector.memset(eps_sb, EPS)

            n_dve = len(DVE_BLOCKS)
            n_act = len(ACT_BLOCKS)
            stats1 = statp.tile([D, max(n_dve, 1), 4, 6], f32)
            mv1 = statp.tile([D, max(n_dve, 1), 2], f32)
            sa = statp.tile([D, max(n_act, 1), 2], f32)
            qa = statp.tile([D, max(n_act, 1), 2], f32)
            arpay1a = statp.tile([D, P], f32)
            arpay1b = statp.tile([D, P], f32)
            arpay2q = [statp.tile([D, 2 * (hi - lo)], f32, name=f"arpay2q{q}")
                       for q, (lo, hi) in enumerate(SEGS)]
            red1a = statp.tile([D, P], f32)
            red1b = statp.tile([D, P], f32)
            red2q = [statp.tile([D, 2 * (hi - lo)], f32, name=f"red2q{q}")
                     for q, (lo, hi) in enumerate(SEGS)]
            gath1a = statp.tile([D, NCORES, P], f32)
            gath1b = statp.tile([D, NCORES, P], f32)
            gath2q = [statp.tile([D, NCORES, 2 * (hi - lo)], f32, name=f"gath2q{q}")
                      for q, (lo, hi) in enumerate(SEGS)]
            Mt = statp.tile([D, P], f32)
            Qt = statp.tile([D, P], f32)
            vt = statp.tile([D, P], f32)
            nrx = statp.tile([D, P], f32)
            nrr = statp.tile([D, P], f32)
            nrt = statp.tile([D, P], f32)
            s1 = statp.tile([D, P], f32)
            t1 = statp.tile([D, P], f32)
            s3 = statp.tile([D, P], f32)
            t3 = statp.tile([D, P], f32)
            su2 = statp.tile([D, P], f32)      # STT1 accums (sum u), col p
            sq2 = statp.tile([D, P], f32)      # sumsq accums, col p

            xu = []
            for p in range(P):
                t = xup.tile([D, BC], bf16, tag=f"xu{p}")
                xu.append(t)

            xr_tiles = {}

            def issue_xr(gi):
                if gi >= NG:
                    return
                t = xrp.tile([D, BC // D, D + 4], fp8, tag="xr")
                xr_tiles[gi] = t
                # col D of each chunk = 1.0 so one 129-wide matmul yields
                # both the gram chunk and the row-sum column
                nc.vector.memset(t[:, :, D:D + 1], 1.0)
                nc.sync.dma_start(
                    t[:, :, 0:D],
                    xr[:, gi * D:(gi + 1) * D].rearrange("(c i) d -> i c d", i=D))

            def wcol(w_sb, p):
                return w_sb[:, p * D:(p + 1) * D]

            def all_gather(arpay, gath, red, tagn):
                npay = arpay.shape[-1]
                agin = dram.tile([D, npay], f32, tag=f"agin{tagn}", name=f"agin{tagn}")
                agout = dram.tile([NCORES * D, npay], f32, tag=f"agout{tagn}",
                                  name=f"agout{tagn}")
                nc.sync.dma_start(agin, arpay)
                nc.gpsimd.collective_compute(
                    "AllGather", mybir.AluOpType.bypass,
                    replica_groups=[list(range(NCORES))],
                    ins=[agin.opt()], outs=[agout.opt()],
                )
                nc.sync.dma_start(gath, agout.rearrange("(r i) f -> i r f", r=NCORES))
                nc.vector.tensor_reduce(out=red, in_=gath[:].rearrange("i r f -> i f r"),
                                        axis=AX, op=add)

            def affine1(red, lo):
                """s1/t1 cols lo:lo+16 from a half-payload [16 mean | 16 E2].
                rstd via ACT Sqrt (pre-tanh, so the table swap is free-ish)."""
                sl = slice(lo, lo + 16)
                nc.vector.tensor_scalar_mul(Mt[:, sl], red[:, 0:16], 1.0 / NCORES)
                nc.vector.tensor_scalar_mul(Qt[:, sl], red[:, 16:32], 1.0 / NCORES)
                nc.vector.tensor_tensor(vt[:, sl], Mt[:, sl], Mt[:, sl], op=mult)
                nc.vector.tensor_tensor(vt[:, sl], Qt[:, sl], vt[:, sl], op=subtract)
                nc.scalar.activation(out=vt[:, sl], in_=vt[:, sl], func=Sqrt,
                                     bias=eps_sb)
                nc.vector.reciprocal(nrr[:, sl], vt[:, sl])
                nc.vector.tensor_tensor(s1[:, sl], g1_sb[:, sl], nrr[:, sl], op=mult)
                nc.vector.tensor_tensor(t1[:, sl], Mt[:, sl], s1[:, sl], op=mult)
                nc.vector.tensor_tensor(t1[:, sl], bt1_sb[:, sl], t1[:, sl], op=subtract)

            def rsqrt_nr(lo, w):
                """nrr[:, lo:lo+w] = 1/sqrt(vt[:, lo:lo+w] + EPS), 2-step
                Newton for sqrt on DVE (var(u) is within ~[0.5, 2.5])."""
                sl = slice(lo, lo + w)
                u = vt[:, sl]
                nc.vector.tensor_scalar_add(u, u, EPS)
                nc.vector.tensor_scalar(nrx[:, sl], u, 0.5, 0.5, mult, add)
                for _ in range(2):
                    nc.vector.reciprocal(nrr[:, sl], nrx[:, sl])
                    nc.vector.tensor_tensor(nrt[:, sl], u, nrr[:, sl], op=mult)
                    nc.vector.tensor_tensor(nrx[:, sl], nrx[:, sl], nrt[:, sl], op=add)
                    nc.vector.tensor_scalar_mul(nrx[:, sl], nrx[:, sl], 0.5)
                nc.vector.reciprocal(nrr[:, sl], nrx[:, sl])

            def payload_half(arpay, gram_l, dve_l, act_l, dve_off, act_off):
                """arpay [D,32] = [means | E2s] in grouped order for one half."""
                h2 = 16
                o = 0
                ng = len(gram_l)
                if ng:
                    g0 = GCOL[gram_l[0]]
                    nc.vector.tensor_scalar_mul(
                        arpay[:, o:o + ng], psMean[:, g0:g0 + ng], 1.0 / BC)
                    nc.vector.tensor_scalar_mul(
                        arpay[:, h2 + o:h2 + o + ng],
                        psMean[:, 32 + g0:32 + g0 + ng], 1.0 / BC)
                    o += ng
                nd = len(dve_l)
                if nd:
                    j0 = dve_off
                    nc.vector.tensor_copy(arpay[:, o:o + nd], mv1[:, j0:j0 + nd, 0])
                    nc.vector.tensor_tensor(arpay[:, h2 + o:h2 + o + nd],
                                            mv1[:, j0:j0 + nd, 0],
                                            mv1[:, j0:j0 + nd, 0], op=mult)
                    nc.vector.tensor_tensor(arpay[:, h2 + o:h2 + o + nd],
                                            arpay[:, h2 + o:h2 + o + nd],
                                            mv1[:, j0:j0 + nd, 1], op=add)
                    o += nd
                na = len(act_l)
                if na:
                    j0 = act_off
                    nc.vector.tensor_reduce(out=arpay[:, o:o + na],
                                            in_=sa[:, j0:j0 + na], axis=AX, op=add)
                    nc.vector.tensor_reduce(out=arpay[:, h2 + o:h2 + o + na],
                                            in_=qa[:, j0:j0 + na], axis=AX, op=add)
                    nc.vector.tensor_scalar_mul(arpay[:, o:o + na],
                                                arpay[:, o:o + na], 1.0 / BC)
                    nc.vector.tensor_scalar_mul(arpay[:, h2 + o:h2 + o + na],
                                                arpay[:, h2 + o:h2 + o + na], 1.0 / BC)

            # ================= Stage A (own PSUM scope) =================
            with (
                tc.tile_pool(name="pHA", bufs=2, space="PSUM") as pHA,
                tc.tile_pool(name="psc", bufs=2, space="PSUM") as pscp,
                tc.tile_pool(name="psm", bufs=1, space="PSUM") as psmp,
                tc.tile_pool(name="pstat", bufs=1, space="PSUM") as pstatp,
            ):
                psMean = pstatp.tile([D, 64], f32)  # cols 0:32 mean-sums, 32:64 E2

                # PE warm-up while first DMAs stream in
                for i in range(8):
                    pw = pHA.tile([D, HW], f32, tag="h")
                    nc.tensor.matmul(pw[:, 0:512], lhsT=w1_sb[:, 0:D],
                                     rhs=w1_sb[:, 0:512], start=True, stop=True)
                    nc.tensor.matmul(pw[:, 512:HW], lhsT=w1_sb[:, 0:D],
                                     rhs=w1_sb[:, 512:HW], start=True, stop=True)

                for gi in range(3):
                    issue_xr(gi)
                for p in ACT_A + DVE_A:
                    nc.sync.dma_start(xu[p], xt[p * D:(p + 1) * D, :])

                def gram_block(p):
                    gi = GCOL[p]
                    issue_xr(gi + 3)
                    xrt = xr_tiles[gi]
                    psC = pscp.tile([D, 132], f32, tag="c")
                    for c in range(BC // D):
                        nc.tensor.matmul(psC[:, 0:D + 1], lhsT=xrt[:, c, 0:D],
                                         rhs=xrt[:, c, 0:D + 1],
                                         start=(c == 0), stop=(c == BC // D - 1))
                    csb = csbp.tile([D, 132], bf16, tag="cs")
                    nc.scalar.copy(csb[:, 0:D + 1], psC[:, 0:D + 1])
                    psM = psmp.tile([D, D], f32, tag="m")
                    nc.tensor.matmul(psM, lhsT=csb[:, 0:D], rhs=wcol(w1_sb, p),
                                     start=True, stop=True)
                    prod = csbp.tile([D, D], bf16, tag="pr")
                    nc.vector.tensor_tensor(prod, wcol(w1_sb, p), psM, op=mult)
                    nc.tensor.matmul(psMean[:, 32 + gi:32 + gi + 1], lhsT=prod,
                                     rhs=ones_sb, start=True, stop=True)
                    nc.tensor.matmul(psMean[:, gi:gi + 1], lhsT=wcol(w1_sb, p),
                                     rhs=csb[:, D:D + 1], start=True, stop=True)

                def y1a_block(p):
                    halves = []
                    for h in range(NH):
                        ps = pHA.tile([D, HW], f32, tag="h")
                        halves.append(ps)
                        for q in range(2):
                            qs = slice(q * 512, (q + 1) * 512)
                            nc.tensor.matmul(ps[:, qs], lhsT=wcol(w1_sb, p),
                                             rhs=xu[p][:, h * HW + q * 512:
                                                       h * HW + (q + 1) * 512],
                                             start=True, stop=True)
                    if p in DVE_BLOCKS:
                        j = DVE_BLOCKS.index(p)
                        for h in range(NH):
                            nc.vector.bn_stats(out=stats1[:, j, 2 * h],
                                               in_=halves[h][:, 0:512])
                            nc.vector.bn_stats(out=stats1[:, j, 2 * h + 1],
                                               in_=halves[h][:, 512:HW])
                        nc.vector.bn_aggr(out=mv1[:, j], in_=stats1[:, j])
                    else:
                        j = ACT_BLOCKS.index(p)
                        for h in range(NH):
                            scr = scrp.tile([D, HW], bf16, tag="scr")
                            nc.scalar.activation(out=scr, in_=halves[h], func=Copy,
                                                 accum_out=sa[:, j, h:h + 1])
                            nc.scalar.activation(out=scr, in_=halves[h], func=Square,
                                                 accum_out=qa[:, j, h:h + 1])

                for p in GRAM_A:
                    gram_block(p)
                for p in ACT_A + DVE_A:
                    y1a_block(p)
                payload_half(arpay1a, GRAM_A, DVE_A, ACT_A, 0, 0)
                all_gather(arpay1a, gath1a, red1a, "1a")

                for p in ACT_B + DVE_B:
                    nc.sync.dma_start(xu[p], xt[p * D:(p + 1) * D, :])
                for p in GRAM_B:
                    gram_block(p)
                for p in ACT_B + DVE_B:
                    y1a_block(p)
                payload_half(arpay1b, GRAM_B, DVE_B, ACT_B, NDA, NAA)
                all_gather(arpay1b, gath1b, red1b, "1b")

                for p in GRAM_BLOCKS:
                    nc.sync.dma_start(xu[p], xt[p * D:(p + 1) * D, :])

            affine1(red1a, 0)
            affine1(red1b, 16)

            # ================= Stage B + C (all 8 PSUM banks) =================
            with tc.tile_pool(name="pH2", bufs=2, space="PSUM") as pH2:

                def affine2(red, lo, w):
                    sl = slice(lo, lo + w)
                    nc.vector.tensor_scalar_mul(Mt[:, sl], red[:, 0:w], 1.0 / NCORES)
                    nc.vector.tensor_scalar_mul(Qt[:, sl], red[:, w:2 * w], 1.0 / NCORES)
                    nc.vector.tensor_tensor(vt[:, sl], Mt[:, sl], Mt[:, sl], op=mult)
                    nc.vector.tensor_tensor(vt[:, sl], Qt[:, sl], vt[:, sl],
                                            op=subtract)
                    rsqrt_nr(lo, w)
                    nc.vector.tensor_tensor(s3[:, sl], g3_sb[:, sl], nrr[:, sl],
                                            op=mult)
                    nc.vector.tensor_tensor(t3[:, sl], Mt[:, sl], s3[:, sl], op=mult)
                    nc.vector.tensor_tensor(t3[:, sl], bt3_sb[:, sl], t3[:, sl],
                                            op=subtract)

                def emit_seg(q):
                    # stage-C flow for one sync-2 segment, interleaved into
                    # stage B so the ScalarE tail stays short
                    lo, hi = SEGS[q]
                    affine2(red2q[q], lo, hi - lo)
                    for pp in range(lo, hi):
                        of = ofp.tile([D, BC], bf16, tag="of", name="of")
                        nc.scalar.activation(out=of, in_=xu[pp], func=Tanh,
                                             bias=t3[:, pp:pp + 1],
                                             scale=s3[:, pp:pp + 1])
                        nc.sync.dma_start(out[pp * D:(pp + 1) * D, :], of)

                GATHER_AT = {hi - 1: q for q, (lo, hi) in enumerate(SEGS)}
                EMIT_AT = {15: [0], 23: [1], 27: [2], 31: [3, 4]}

                for p in range(P):
                    c1 = COL1[p]
                    o1 = o1p.tile([D, BC], bf16, tag="o1")
                    ys = pH2.tile([D, BC], f32, tag="y")
                    for q in range(4):
                        nc.tensor.matmul(ys[:, q * 512:(q + 1) * 512],
                                         lhsT=wcol(w1_sb, p),
                                         rhs=xu[p][:, q * 512:(q + 1) * 512],
                                         start=True, stop=True)
                    nc.scalar.activation(out=o1, in_=ys, func=Tanh,
                                         bias=t1[:, c1:c1 + 1], scale=s1[:, c1:c1 + 1])
                    us = ys  # mm2 overwrites the y1 tile after tanh consumed it
                    for q in range(4):
                        gs = slice(q * 512, (q + 1) * 512)
                        nc.tensor.matmul(us[:, gs], lhsT=wcol(w2_sb, p),
                                         rhs=o1[:, gs], start=True, stop=True)
                    nc.vector.scalar_tensor_tensor(
                        out=xu[p], in0=us, scalar=1.0, in1=xu[p],
                        op0=mult, op1=add, accum_out=su2[:, p:p + 1])
                    scr = scrp.tile([D, BC], bf16, tag="scr")
                    if p in SQ_ACT:
                        nc.scalar.activation(out=scr, in_=xu[p], func=Square,
                                             accum_out=sq2[:, p:p + 1])
                    else:
                        nc.vector.scalar_tensor_tensor(
                            out=scr, in0=xu[p], scalar=1.0, in1=xu[p],
                            op0=mult, op1=mult, accum_out=sq2[:, p:p + 1])

                    if p in GATHER_AT:
                        q = GATHER_AT[p]
                        lo, hi = SEGS[q]
                        w = hi - lo
                        nc.vector.tensor_scalar_mul(arpay2q[q][:, 0:w],
                                                    su2[:, lo:hi], 1.0 / BC)
                        nc.vector.tensor_scalar_mul(arpay2q[q][:, w:2 * w],
                                                    sq2[:, lo:hi], 1.0 / BC)
                        all_gather(arpay2q[q], gath2q[q], red2q[q], f"2q{q}")
                    for q in EMIT_AT.get(p, []):
                        emit_seg(q)

    return nc


def _get_nc():
    if "nc" not in _state:
        _install_tile_drain_patch()
        _install_ntff_hook()
        _state["nc"] = _build()
    return _state["nc"]


def kernel(x, weights1, bias1, weights2, bias2, gamma1, beta1, gamma3, beta3):
    from concourse.bass_utils import run_bass_kernel_spmd

    x = np.asarray(x, dtype=np.float32)
    w1 = np.asarray(weights1, dtype=np.float32)
    w2 = np.asarray(weights2, dtype=np.float32)
    gamma1 = np.asarray(gamma1, dtype=np.float32)
    beta1 = np.asarray(beta1, dtype=np.float32)
    gamma3 = np.asarray(gamma3, dtype=np.float32)
    beta3 = np.asarray(beta3, dtype=np.float32)

    nc = _get_nc()

    x_bf = x.astype(_BF16)                                  # [B, F]
    xT = np.ascontiguousarray(x_bf.T)                       # [F, B]
    gsel = np.concatenate([np.arange(p * D, (p + 1) * D) for p in GRAM_BLOCKS])
    w1h = np.ascontiguousarray(w1.transpose(1, 0, 2).reshape(D, F)).astype(_BF16)
    w2h = np.ascontiguousarray(w2.transpose(1, 0, 2).reshape(D, F)).astype(_BF16)
    perm = np.asarray(GROUPED)
    g1h = np.ascontiguousarray(gamma1.reshape(P, D).T[:, perm])
    bt1h = np.ascontiguousarray(beta1.reshape(P, D).T[:, perm])
    g3h = np.ascontiguousarray(gamma3.reshape(P, D).T)
    bt3h = np.ascontiguousarray(beta3.reshape(P, D).T)

    in_maps = []
    for cid in range(NCORES):
        rows = slice(cid * BC, (cid + 1) * BC)
        in_maps.append({
            "xt": np.ascontiguousarray(xT[:, rows]),
            "xr": np.ascontiguousarray(x[rows][:, gsel]).astype(_FP8),
            "w1": w1h, "w2": w2h,
            "g1": g1h, "bt1": bt1h, "g3": g3h, "bt3": bt3h,
        })

    res = run_bass_kernel_spmd(nc, in_maps, core_ids=list(range(NCORES)))
    _state["last_exec_time_ns"] = res.exec_time_ns

    outT = np.empty((B, F), dtype=np.float32)
    for cid in range(NCORES):
        outT[cid * BC:(cid + 1) * BC, :] = res.results[cid]["out"].T.astype(np.float32)
    return outT
